# revision 1
# baseline (speedup 1.0000x reference)
"""Per-pixel predicted 5x5 conv (KPN-style) on 8 trn2 cores.

Sharding: data-parallel over (batch x H-half) = 8 shards, halo rows included
in each shard's input slice (host-side zero-padded, so no edge cases).

Device layout (per core):
  partitions = 128 output rows (h), free = (c, w) c-major.
  - 10 SBUF copies of the feat slice: 5 h-shifts (di) x 2 w-parities, so every
    tap (di, dj) is a clean slice with 4B-aligned, stride-1 inner w runs ->
    DVE tensor_tensor runs in 2x_1P bf16 mode.
  - per tap: DVE bf16 multiply prod = feat_shift * kernel_tap (kernel tap
    broadcast across c via stride-0 AP dim).
  - 25-tap accumulation: PE identity-matmul PSUM accumulate (start on a bias
    matmul, so bias rides along for free).
  - ACT evacuates PSUM -> SBUF fp32, DMA out.
"""

import sys

for p in ("/opt/pypackages", "/opt/trn_rl_repo"):
    if p not in sys.path:
        sys.path.insert(0, p)

import numpy as np
import ml_dtypes

import concourse.mybir as mybir
from concourse import bacc, tile
from concourse.bass_utils import run_bass_kernel_spmd

B, H, W, C, KK, K = 4, 256, 256, 32, 25, 5
HS = H // 2          # 128 output rows per core
WPAD = W + 8         # w index j == original w (j-2); zeros outside
CH = 16              # channels per half-pass (SBUF fit)
CQ = 8               # channels per PSUM chunk (4 banks)
BF16 = mybir.dt.bfloat16
F32 = mybir.dt.float32

_NC_CACHE = {}


def _build_nc():
    nc = bacc.Bacc(None, target_bir_lowering=False)
    feat_d = nc.dram_tensor("feat", [HS + 4, C, WPAD], BF16, kind="ExternalInput")
    kern_d = nc.dram_tensor("kern", [HS, KK, W], BF16, kind="ExternalInput")
    bias_d = nc.dram_tensor("biasr", [128, C, W], BF16, kind="ExternalInput")
    iden_d = nc.dram_tensor("iden", [128, 128], BF16, kind="ExternalInput")
    out_d = nc.dram_tensor("out", [HS, C, W], F32, kind="ExternalOutput")

    with tile.TileContext(nc) as tc:
        with tc.tile_pool(name="const", bufs=1) as cpool, \
             tc.tile_pool(name="copies", bufs=2) as fpool, \
             tc.tile_pool(name="prod", bufs=6) as ppool, \
             tc.tile_pool(name="osb", bufs=4) as opool, \
             tc.tile_pool(name="psum", bufs=2, space="PSUM") as qpool:
            ident = cpool.tile([128, 128], BF16, tag="ident")
            nc.sync.dma_start(out=ident, in_=iden_d[:, :])
            kern_t = cpool.tile([128, KK, W], BF16, tag="kern")
            nc.sync.dma_start(out=kern_t, in_=kern_d[:, :, :])
            bias_t = cpool.tile([128, C, W], BF16, tag="bias")
            nc.sync.dma_start(out=bias_t, in_=bias_d[:, :, :])

            for qp in range(C // CQ):          # quarter-pass = one PSUM chunk
                cq0 = qp * CQ
                cops = {}
                for di in range(K):
                    for par in range(2):
                        t = fpool.tile([128, CQ, W + 4], BF16,
                                       tag=f"cop{di}_{par}")
                        nc.sync.dma_start(
                            out=t,
                            in_=feat_d[di:di + 128, cq0:cq0 + CQ,
                                       par:par + W + 4])
                        cops[(di, par)] = t
                psum_t = qpool.tile([128, 4, 512], F32, tag="ps")
                # bias seeds the accumulation group (start=True)
                for j in range(4):
                    nc.tensor.matmul(
                        psum_t[:, j:j + 1, :],
                        ident,
                        bias_t[:, cq0 + 2 * j:cq0 + 2 * j + 2, :],
                        start=True, stop=False)
                for ti in range(KK):
                    di, dj = ti // K, ti % K
                    par = dj % 2
                    s = dj - par
                    cop = cops[(di, par)]
                    prod = ppool.tile([128, CQ, W], BF16, tag="prod")
                    in0 = cop[:, :, s:s + W]
                    in1 = kern_t[:, ti:ti + 1, :].broadcast_to(
                        (128, CQ, W))
                    nc.vector.tensor_tensor(prod, in0, in1,
                                            mybir.AluOpType.mult)
                    last = ti == KK - 1
                    for j in range(4):
                        nc.tensor.matmul(
                            psum_t[:, j:j + 1, :],
                            ident,
                            prod[:, 2 * j:2 * j + 2, :],
                            start=False, stop=last)
                for j in range(4):
                    out_sb = opool.tile([128, 2, W], F32, tag="osb")
                    nc.scalar.copy(
                        out=out_sb.rearrange("p a b -> p (a b)"),
                        in_=psum_t[:, j:j + 1, :].rearrange(
                            "p a b -> p (a b)"))
                    nc.sync.dma_start(
                        out=out_d[:, cq0 + 2 * j:cq0 + 2 * j + 2, :],
                        in_=out_sb)
    if not nc.is_finalized():
        nc.finalize()
    return nc


def _get_nc():
    if "nc" not in _NC_CACHE:
        _NC_CACHE["nc"] = _build_nc()
    return _NC_CACHE["nc"]


def _prep_inputs(feat, kernel, bias):
    ft = np.ascontiguousarray(feat.transpose(0, 1, 3, 2))   # [B, H, C, W]
    fp = np.zeros((B, H + 4, C, WPAD), np.float32)
    fp[:, 2:H + 2, :, 2:W + 2] = ft
    fpb = fp.astype(ml_dtypes.bfloat16)
    kt = np.ascontiguousarray(
        kernel.transpose(0, 1, 3, 2)).astype(ml_dtypes.bfloat16)  # [B,H,25,W]
    biasr = np.ascontiguousarray(
        np.broadcast_to(
            bias.astype(ml_dtypes.bfloat16)[None, :, None], (128, C, W)))
    iden = np.eye(128, dtype=ml_dtypes.bfloat16)
    in_maps = []
    for core in range(8):
        b, hh = core // 2, core % 2
        h0 = hh * HS
        in_maps.append({
            "feat": np.ascontiguousarray(fpb[b, h0:h0 + HS + 4]),
            "kern": np.ascontiguousarray(kt[b, h0:h0 + HS]),
            "biasr": biasr,
            "iden": iden,
        })
    return in_maps


def _run(feat, kernel, bias, **run_kwargs):
    nc = _get_nc()
    in_maps = _prep_inputs(feat, kernel, bias)
    res = run_bass_kernel_spmd(nc, in_maps, core_ids=list(range(8)),
                               **run_kwargs)
    out = np.empty((B, H, C, W), np.float32)
    for core in range(8):
        b, hh = core // 2, core % 2
        out[b, hh * HS:(hh + 1) * HS] = res.results[core]["out"]
    return np.ascontiguousarray(out.transpose(0, 1, 3, 2)), res


def kernel(feat, kernel, bias):
    out, _ = _run(np.asarray(feat, np.float32), np.asarray(kernel, np.float32),
                  np.asarray(bias, np.float32))
    return out



# revision 28
# speedup vs baseline: 1.3306x; 1.3306x over previous
"""Per-pixel predicted 5x5 conv (KPN-style) on 8 trn2 cores.

Sharding: data-parallel over (batch x H-half) = 8 shards; each core gets 128
output rows plus a 4-row halo (host-prepared, zero-padded at image edges).

Device layout (per core), partitions = output row h, free = (c, w) c-major:
  - 5 SBUF copies of the feat slice, one per h-shift di (the +-2 row halo is
    absorbed by the copy offsets). w-shifts dj are plain free-dim offsets.
  - per tap: bf16 multiply prod = feat_shift * kernel_tap (tap broadcast
    across c via a stride-0 AP dim). ~20 taps/pass on DVE, ~5 on GPSIMD
    (Pool) to balance the two engines.
  - 25-tap accumulation: PE identity-matmul PSUM accumulate; the group is
    seeded (start=True) by a bias matmul so the bias rides along for free.
  - one ACT op per pass evacuates PSUM -> SBUF as bf16, DMA out.
C=32 is processed in 4 passes of 8 channels (one PSUM half each, double
buffered).
"""

import sys

for p in ("/opt/pypackages", "/opt/trn_rl_repo"):
    if p not in sys.path:
        sys.path.insert(0, p)

import numpy as np
import ml_dtypes

import concourse.mybir as mybir
from concourse import bacc, tile
from concourse.ap import AP
from concourse.bass_utils import run_bass_kernel_spmd

B, H, W, C, KK, K = 4, 256, 256, 32, 25, 5
HS = H // 2          # 128 output rows per core
WPAD = W + 8         # feat w index = w + 2 (zeros outside)
CQ = 8               # channels per pass (4 PSUM banks)
NQP = C // CQ
BF16 = mybir.dt.bfloat16
F32 = mybir.dt.float32

# taps routed to the Pool (GPSIMD) engine; the rest go to DVE.  Spread so the
# PE meets each Pool product roughly when it is ready.
POOL_TAPS = (4, 9, 14, 19, 24)

_NC_CACHE = {}


def _build_nc():
    nc = bacc.Bacc(None, target_bir_lowering=False)
    feat_d = nc.dram_tensor("feat", [HS + 4, C, WPAD], BF16, kind="ExternalInput")
    kern_d = nc.dram_tensor("kern", [HS, KK, W], BF16, kind="ExternalInput")
    bias_d = nc.dram_tensor("biasr", [128, C * W], BF16, kind="ExternalInput")
    iden_d = nc.dram_tensor("iden", [128, 128], BF16, kind="ExternalInput")
    out_d = nc.dram_tensor("out", [HS, C, W], BF16, kind="ExternalOutput")

    with tile.TileContext(nc) as tc:
        with tc.tile_pool(name="const", bufs=1) as cpool, \
             tc.tile_pool(name="dprod", bufs=6) as dpool, \
             tc.tile_pool(name="pprod", bufs=6) as gpool, \
             tc.tile_pool(name="osb", bufs=2) as opool, \
             tc.tile_pool(name="accp", bufs=2) as apool, \
             tc.tile_pool(name="psum", bufs=2, space="PSUM") as qpool:
            # kern in three pieces, interleaved with the first feat slices,
            # so each di group's taps arrive just ahead of the compute
            kern_a = cpool.tile([128, 5, W], BF16, tag="kerna")
            nc.sync.dma_start(out=kern_a, in_=kern_d[:, 0:5, :])

            # feat slices: one tile per (di, qp) so deps are exact
            cops = [[None] * NQP for _ in range(K)]

            def load_cop(di, qp):
                t = cpool.tile([128, CQ, WPAD], BF16, tag=f"cop{di}_{qp}")
                nc.sync.dma_start(
                    out=t, in_=feat_d[di:di + 128, qp * CQ:(qp + 1) * CQ, :])
                cops[di][qp] = t

            # bias replicated over partitions, loaded per qp-slice
            bias_q = []
            for qp in range(NQP):
                bq = cpool.tile([128, CQ * W], BF16, tag=f"bias{qp}")
                bias_q.append(bq)

            def load_bias(qp):
                nc.sync.dma_start(
                    out=bias_q[qp],
                    in_=bias_d[:, qp * CQ * W:(qp + 1) * CQ * W])

            load_cop(0, 0)
            ident = cpool.tile([128, 128], BF16, tag="ident")
            nc.sync.dma_start(out=ident, in_=iden_d[:, :])
            load_bias(0)
            load_cop(1, 0)
            kern_b = cpool.tile([128, 5, W], BF16, tag="kernb")
            nc.sync.dma_start(out=kern_b, in_=kern_d[:, 5:10, :])
            load_cop(2, 0)
            kern_c = cpool.tile([128, 15, W], BF16, tag="kernc")
            nc.sync.dma_start(out=kern_c, in_=kern_d[:, 10:25, :])
            load_cop(3, 0)
            load_cop(4, 0)
            for qp in range(1, NQP):
                for di in range(K):
                    load_cop(di, qp)
                load_bias(qp)

            def kern_slice(ti, n):
                if ti < 5:
                    return kern_a[:, ti:ti + n, :]
                if ti < 10:
                    return kern_b[:, ti - 5:ti - 5 + n, :]
                return kern_c[:, ti - 10:ti - 10 + n, :]

            def kern_pair(ti):
                # broadcast two adjacent taps over c: [128, 2, CQ, W]
                t = kern_slice(ti, 2)
                return AP(tensor=t.tensor, offset=t.offset,
                          ap=[list(t.ap[0]), [W, 2], [0, CQ], [1, W]])

            def kern_one(ti):
                return kern_slice(ti, 1).broadcast_to((128, CQ, W))

            # bias seeds each PSUM via start=True identity matmuls (the
            # hardware-reliable way to initialize an accumulation group).
            psums = [None] * NQP

            def psum_init(qp):
                psa = qpool.tile([128, 2, 512], F32, tag="psa")
                psb = qpool.tile([128, 2, 512], F32, tag="psb")
                psums[qp] = (psa, psb)
                for j in range(4):
                    ps = (psa, psb)[j // 2]
                    nc.tensor.matmul(
                        ps[:, j % 2:j % 2 + 1, :], ident,
                        bias_q[qp][:, j * 512:(j + 1) * 512],
                        start=True, stop=False, skip_group_check=True)

            psum_init(0)
            psum_init(1)
            def mm(psum_pair, j, moving, stop):
                ps = psum_pair[j // 2]
                nc.tensor.matmul(
                    ps[:, j % 2:j % 2 + 1, :], ident, moving,
                    start=False, stop=stop, skip_group_check=True)

            for qp in range(NQP):
                cq0 = qp * CQ
                psum_t = psums[qp]
                acc = None
                for di in range(K):
                    finale = qp == NQP - 1 and di == K - 1

                    def dve_pair(pj, stop=False):
                        dj = 2 * pj
                        ti = di * K + dj
                        cop = cops[di][qp]
                        in0 = AP(tensor=cop.tensor, offset=cop.offset + dj,
                                 ap=[list(cop.ap[0]), [1, 2], [WPAD, CQ],
                                     [1, W]])
                        prod = dpool.tile([128, 2, CQ, W], BF16, tag="dprod")
                        nc.vector.tensor_tensor(prod, in0, kern_pair(ti),
                                                mybir.AluOpType.mult)
                        if stop:
                            # j-major so each psum half closes early
                            for j in range(4):
                                for t2 in range(2):
                                    mm(psum_t, j, prod[:, t2, 2 * j:2 * j + 2, :],
                                       t2 == 1)
                        else:
                            for t2 in range(2):
                                for j in range(4):
                                    mm(psum_t, j, prod[:, t2, 2 * j:2 * j + 2, :],
                                       False)

                    def pool_one():
                        # di 0: product lands in the qp accumulator; di 1:
                        # product is DMA-accumulated onto it (frees PE work;
                        # the merged set is consumed at the end of the pass,
                        # giving the DMA plenty of slack); di >= 2: straight
                        # to PE.
                        ti = di * K + 4
                        in0 = cops[di][qp][:, :, 4:4 + W]
                        if di == 0:
                            prodp = apool.tile([128, CQ, W], BF16, tag="acc")
                            nc.gpsimd.tensor_tensor(prodp, in0, kern_one(ti),
                                                    mybir.AluOpType.mult)
                            return prodp
                        prodp = gpool.tile([128, CQ, W], BF16, tag="pprod")
                        nc.gpsimd.tensor_tensor(prodp, in0, kern_one(ti),
                                                mybir.AluOpType.mult)
                        if di == 1:
                            nc.gpsimd.dma_start(out=acc, in_=prodp,
                                                accum_op=mybir.AluOpType.add)
                            return None
                        for j in range(4):
                            mm(psum_t, j, prodp[:, 2 * j:2 * j + 2, :], False)
                        return None

                    if finale:
                        # merged acc set first (ready long ago); the whole
                        # last group runs on DVE so the PE never goes idle
                        # waiting for a late Pool product.
                        for j in range(4):
                            mm(psum_t, j, acc[:, 2 * j:2 * j + 2, :], False)
                        dve_pair(0)
                        dve_pair(1)
                        ti = di * K + 4
                        half = CQ // 2
                        for hb in range(2):
                            in0s = cops[di][qp][:, hb * half:(hb + 1) * half,
                                                4:4 + W]
                            prods = apool.tile([128, half, W], BF16,
                                               tag=f"psng{hb}")
                            kap = kern_slice(ti, 1)
                            k1 = AP(tensor=kap.tensor, offset=kap.offset,
                                    ap=[list(kap.ap[0]), [0, half], [1, W]])
                            nc.vector.tensor_tensor(prods, in0s, k1,
                                                    mybir.AluOpType.mult)
                            for j in range(2 * hb, 2 * hb + 2):
                                mm(psum_t, j,
                                   prods[:, (j - 2 * hb) * 2:
                                         (j - 2 * hb) * 2 + 2, :],
                                   j == 2 * hb + 1)
                    else:
                        r = pool_one()
                        if r is not None:
                            acc = r
                        if di == K - 1:
                            # consume the merged pool accumulator just before
                            # the last DVE group (plenty of DMA slack, and the
                            # PE never ends a pass waiting on it)
                            for j in range(4):
                                mm(psum_t, j, acc[:, 2 * j:2 * j + 2, :],
                                   False)
                        dve_pair(0)
                        dve_pair(1, stop=di == K - 1)
                out_sb = opool.tile([128, CQ, W], BF16, tag="osb")
                nc.scalar.copy(
                    out=out_sb[:, 0:CQ // 2, :].rearrange("p a b -> p (a b)"),
                    in_=psum_t[0].rearrange("p a b -> p (a b)"))
                nc.sync.dma_start(
                    out=out_d[:, cq0:cq0 + CQ // 2, :],
                    in_=out_sb[:, 0:CQ // 2, :])
                nc.scalar.copy(
                    out=out_sb[:, CQ // 2:CQ, :].rearrange("p a b -> p (a b)"),
                    in_=psum_t[1].rearrange("p a b -> p (a b)"))
                if qp + 2 < NQP:
                    psum_init(qp + 2)
                nc.sync.dma_start(
                    out=out_d[:, cq0 + CQ // 2:cq0 + CQ, :],
                    in_=out_sb[:, CQ // 2:CQ, :])
    if not nc.is_finalized():
        nc.finalize()
    return nc


def _get_nc():
    if "nc" not in _NC_CACHE:
        _NC_CACHE["nc"] = _build_nc()
    return _NC_CACHE["nc"]


def _prep_inputs(feat, kernel, bias):
    ft = np.ascontiguousarray(feat.transpose(0, 1, 3, 2))   # [B, H, C, W]
    fp = np.zeros((B, H + 4, C, WPAD), np.float32)
    fp[:, 2:H + 2, :, 2:W + 2] = ft
    fpb = fp.astype(ml_dtypes.bfloat16)
    kt = np.ascontiguousarray(
        kernel.transpose(0, 1, 3, 2)).astype(ml_dtypes.bfloat16)  # [B,H,25,W]
    biasr = np.ascontiguousarray(np.broadcast_to(
        np.repeat(bias.astype(np.float32), W).astype(ml_dtypes.bfloat16),
        (128, C * W)))
    iden = np.eye(128, dtype=ml_dtypes.bfloat16)
    in_maps = []
    for core in range(8):
        b, hh = core // 2, core % 2
        h0 = hh * HS
        in_maps.append({
            "feat": np.ascontiguousarray(fpb[b, h0:h0 + HS + 4]),
            "kern": np.ascontiguousarray(kt[b, h0:h0 + HS]),
            "biasr": biasr,
            "iden": iden,
        })
    return in_maps


def _run(feat, kernel, bias, **run_kwargs):
    nc = _get_nc()
    in_maps = _prep_inputs(feat, kernel, bias)
    res = run_bass_kernel_spmd(nc, in_maps, core_ids=list(range(8)),
                               **run_kwargs)
    out = np.empty((B, H, C, W), np.float32)
    for core in range(8):
        b, hh = core // 2, core % 2
        out[b, hh * HS:(hh + 1) * HS] = np.asarray(
            res.results[core]["out"]).astype(np.float32)
    return np.ascontiguousarray(out.transpose(0, 1, 3, 2)), res


def kernel(feat, kernel, bias):
    out, _ = _run(np.asarray(feat, np.float32), np.asarray(kernel, np.float32),
                  np.asarray(bias, np.float32))
    return out


# revision 40
# speedup vs baseline: 1.3326x; 1.0016x over previous
"""Per-pixel predicted 5x5 conv (KPN-style) on 8 trn2 cores.

Sharding: data-parallel over (batch x H-half) = 8 shards; each core gets 128
output rows plus a 4-row halo (host-prepared, zero-padded at image edges).

Device layout (per core), partitions = output row h, free = (c, w) c-major:
  - 5 SBUF copies of the feat slice, one per h-shift di (the +-2 row halo is
    absorbed by the copy offsets). w-shifts dj are plain free-dim offsets.
  - per tap: bf16 multiply prod = feat_shift * kernel_tap (tap broadcast
    across c via a stride-0 AP dim). ~20 taps/pass on DVE, ~5 on GPSIMD
    (Pool) to balance the two engines.
  - 25-tap accumulation: PE identity-matmul PSUM accumulate; the group is
    seeded (start=True) by a bias matmul so the bias rides along for free.
  - one ACT op per pass evacuates PSUM -> SBUF as bf16, DMA out.
C=32 is processed in 4 passes of 8 channels (one PSUM half each, double
buffered).
"""

import sys

for p in ("/opt/pypackages", "/opt/trn_rl_repo"):
    if p not in sys.path:
        sys.path.insert(0, p)

import numpy as np
import ml_dtypes

import concourse.mybir as mybir
from concourse import bacc, tile
from concourse.ap import AP
from concourse.bass_utils import run_bass_kernel_spmd

B, H, W, C, KK, K = 4, 256, 256, 32, 25, 5
HS = H // 2          # 128 output rows per core
WPAD = W + 8         # feat w index = w + 2 (zeros outside)
CQ = 8               # channels per pass (4 PSUM banks)
NQP = C // CQ
BF16 = mybir.dt.bfloat16
F32 = mybir.dt.float32

# taps routed to the Pool (GPSIMD) engine; the rest go to DVE.  Spread so the
# PE meets each Pool product roughly when it is ready.
POOL_TAPS = (4, 9, 14, 19, 24)

_NC_CACHE = {}


def _build_nc():
    nc = bacc.Bacc(None, target_bir_lowering=False)
    feat_d = nc.dram_tensor("feat", [HS + 4, C, WPAD], BF16, kind="ExternalInput")
    kern_d = nc.dram_tensor("kern", [HS, KK, W], BF16, kind="ExternalInput")
    bias_d = nc.dram_tensor("biasr", [128, C * W], BF16, kind="ExternalInput")
    iden_d = nc.dram_tensor("iden", [128, 128], BF16, kind="ExternalInput")
    out_d = nc.dram_tensor("out", [HS, C, W], BF16, kind="ExternalOutput")

    with tile.TileContext(nc) as tc:
        with tc.tile_pool(name="const", bufs=1) as cpool, \
             tc.tile_pool(name="dprod", bufs=3) as dpool, \
             tc.tile_pool(name="pprod", bufs=6) as gpool, \
             tc.tile_pool(name="osb", bufs=2) as opool, \
             tc.tile_pool(name="accp", bufs=2) as apool, \
             tc.tile_pool(name="psum", bufs=2, space="PSUM") as qpool:
            # kern in three pieces, interleaved with the first feat slices,
            # so each di group's taps arrive just ahead of the compute
            kern_a = cpool.tile([128, 5, W], BF16, tag="kerna")
            nc.sync.dma_start(out=kern_a, in_=kern_d[:, 0:5, :])

            # feat slices: one tile per (di, qp) so deps are exact
            cops = [[None] * NQP for _ in range(K)]

            def load_cop(di, qp):
                t = cpool.tile([128, CQ, WPAD], BF16, tag=f"cop{di}_{qp}")
                nc.sync.dma_start(
                    out=t, in_=feat_d[di:di + 128, qp * CQ:(qp + 1) * CQ, :])
                cops[di][qp] = t

            # bias replicated over partitions, loaded per qp-slice
            bias_q = []
            for qp in range(NQP):
                bq = cpool.tile([128, CQ * W], BF16, tag=f"bias{qp}")
                bias_q.append(bq)

            def load_bias(qp):
                nc.sync.dma_start(
                    out=bias_q[qp],
                    in_=bias_d[:, qp * CQ * W:(qp + 1) * CQ * W])

            load_cop(0, 0)
            ident = cpool.tile([128, 128], BF16, tag="ident")
            nc.sync.dma_start(out=ident, in_=iden_d[:, :])
            load_bias(0)
            load_cop(1, 0)
            kern_b = cpool.tile([128, 5, W], BF16, tag="kernb")
            nc.sync.dma_start(out=kern_b, in_=kern_d[:, 5:10, :])
            load_cop(2, 0)
            kern_c = cpool.tile([128, 15, W], BF16, tag="kernc")
            nc.sync.dma_start(out=kern_c, in_=kern_d[:, 10:25, :])
            load_cop(3, 0)
            load_cop(4, 0)
            for qp in range(1, NQP):
                for di in range(K):
                    load_cop(di, qp)
                load_bias(qp)

            def kern_slice(ti, n):
                if ti < 5:
                    return kern_a[:, ti:ti + n, :]
                if ti < 10:
                    return kern_b[:, ti - 5:ti - 5 + n, :]
                return kern_c[:, ti - 10:ti - 10 + n, :]

            def kern_quad(ti):
                # broadcast four adjacent taps over c: [128, 4, CQ, W]
                t = kern_slice(ti, 4)
                return AP(tensor=t.tensor, offset=t.offset,
                          ap=[list(t.ap[0]), [W, 4], [0, CQ], [1, W]])

            def kern_one(ti):
                return kern_slice(ti, 1).broadcast_to((128, CQ, W))

            # bias seeds each PSUM via start=True identity matmuls (the
            # hardware-reliable way to initialize an accumulation group).
            psums = [None] * NQP

            def psum_init(qp):
                psa = qpool.tile([128, 2, 512], F32, tag="psa")
                psb = qpool.tile([128, 2, 512], F32, tag="psb")
                psums[qp] = (psa, psb)
                for j in range(4):
                    ps = (psa, psb)[j // 2]
                    nc.tensor.matmul(
                        ps[:, j % 2:j % 2 + 1, :], ident,
                        bias_q[qp][:, j * 512:(j + 1) * 512],
                        start=True, stop=False, skip_group_check=True)

            psum_init(0)
            psum_init(1)
            def mm(psum_pair, j, moving, stop):
                ps = psum_pair[j // 2]
                nc.tensor.matmul(
                    ps[:, j % 2:j % 2 + 1, :], ident, moving,
                    start=False, stop=stop, skip_group_check=True)

            for qp in range(NQP):
                cq0 = qp * CQ
                psum_t = psums[qp]
                acc = None
                for di in range(K):
                    finale = qp == NQP - 1 and di == K - 1

                    def dve_quad(stop=False):
                        ti = di * K
                        cop = cops[di][qp]
                        in0 = AP(tensor=cop.tensor, offset=cop.offset,
                                 ap=[list(cop.ap[0]), [1, 4], [WPAD, CQ],
                                     [1, W]])
                        prod = dpool.tile([128, 4, CQ, W], BF16, tag="dprod")
                        nc.vector.tensor_tensor(prod, in0, kern_quad(ti),
                                                mybir.AluOpType.mult)
                        if stop:
                            # j-major so each psum half closes early
                            for j in range(4):
                                for t4 in range(4):
                                    mm(psum_t, j, prod[:, t4, 2 * j:2 * j + 2, :],
                                       t4 == 3)
                        else:
                            for t4 in range(4):
                                for j in range(4):
                                    mm(psum_t, j, prod[:, t4, 2 * j:2 * j + 2, :],
                                       False)

                    def pool_one():
                        # di 0: product lands in the qp accumulator; di 1:
                        # product is DMA-accumulated onto it (frees PE work;
                        # the merged set is consumed at the end of the pass,
                        # giving the DMA plenty of slack); di >= 2: straight
                        # to PE.
                        ti = di * K + 4
                        in0 = cops[di][qp][:, :, 4:4 + W]
                        merge = True
                        if merge and di == 0:
                            prodp = apool.tile([128, CQ, W], BF16, tag="acc")
                            nc.gpsimd.tensor_tensor(prodp, in0, kern_one(ti),
                                                    mybir.AluOpType.mult)
                            return prodp
                        prodp = gpool.tile([128, CQ, W], BF16, tag="pprod")
                        nc.gpsimd.tensor_tensor(prodp, in0, kern_one(ti),
                                                mybir.AluOpType.mult)
                        if merge and di == 1:
                            nc.gpsimd.dma_start(out=acc, in_=prodp,
                                                accum_op=mybir.AluOpType.add)
                            return None
                        for j in range(4):
                            mm(psum_t, j, prodp[:, 2 * j:2 * j + 2, :], False)
                        return None

                    if finale:
                        # the whole last group runs on DVE as two pair ops
                        # plus two c-half singles; the merged pool
                        # accumulator (ready well before) closes the pass.
                        for pj in range(2):
                            dj = 2 * pj
                            ti = di * K + dj
                            cop = cops[di][qp]
                            in0p = AP(tensor=cop.tensor,
                                      offset=cop.offset + dj,
                                      ap=[list(cop.ap[0]), [1, 2], [WPAD, CQ],
                                          [1, W]])
                            kt = kern_slice(ti, 2)
                            k2 = AP(tensor=kt.tensor, offset=kt.offset,
                                    ap=[list(kt.ap[0]), [W, 2], [0, CQ],
                                        [1, W]])
                            prodq = dpool.tile([128, 4, CQ, W], BF16,
                                               tag="dprod")
                            prod = prodq[:, 0:2, :, :]
                            nc.vector.tensor_tensor(prod, in0p, k2,
                                                    mybir.AluOpType.mult)
                            for t2 in range(2):
                                for j in range(4):
                                    mm(psum_t, j,
                                       prodq[:, t2, 2 * j:2 * j + 2, :], False)
                        ti = di * K + 4
                        half = CQ // 2
                        for hb in range(2):
                            in0s = cops[di][qp][:, hb * half:(hb + 1) * half,
                                                4:4 + W]
                            prods = apool.tile([128, half, W], BF16,
                                               tag=f"psng{hb}")
                            kap = kern_slice(ti, 1)
                            k1 = AP(tensor=kap.tensor, offset=kap.offset,
                                    ap=[list(kap.ap[0]), [0, half], [1, W]])
                            nc.vector.tensor_tensor(prods, in0s, k1,
                                                    mybir.AluOpType.mult)
                            for j in range(2 * hb, 2 * hb + 2):
                                mm(psum_t, j,
                                   prods[:, (j - 2 * hb) * 2:
                                         (j - 2 * hb) * 2 + 2, :],
                                   False)
                        for j in range(4):
                            mm(psum_t, j, acc[:, 2 * j:2 * j + 2, :], True)
                    else:
                        r = pool_one()
                        if r is not None:
                            acc = r
                        if di == K - 1 and qp != NQP - 1:
                            # consume the merged pool accumulator just before
                            # the last DVE group (plenty of DMA slack, and
                            # the PE never ends a pass waiting on it)
                            for j in range(4):
                                mm(psum_t, j, acc[:, 2 * j:2 * j + 2, :],
                                   False)
                        dve_quad(stop=di == K - 1)
                out_sb = opool.tile([128, CQ, W], BF16, tag="osb")
                nc.scalar.copy(
                    out=out_sb[:, 0:CQ // 2, :].rearrange("p a b -> p (a b)"),
                    in_=psum_t[0].rearrange("p a b -> p (a b)"))
                nc.sync.dma_start(
                    out=out_d[:, cq0:cq0 + CQ // 2, :],
                    in_=out_sb[:, 0:CQ // 2, :])
                nc.scalar.copy(
                    out=out_sb[:, CQ // 2:CQ, :].rearrange("p a b -> p (a b)"),
                    in_=psum_t[1].rearrange("p a b -> p (a b)"))
                if qp + 2 < NQP:
                    psum_init(qp + 2)
                nc.sync.dma_start(
                    out=out_d[:, cq0 + CQ // 2:cq0 + CQ, :],
                    in_=out_sb[:, CQ // 2:CQ, :])
    if not nc.is_finalized():
        nc.finalize()
    return nc


def _get_nc():
    if "nc" not in _NC_CACHE:
        _NC_CACHE["nc"] = _build_nc()
    return _NC_CACHE["nc"]


def _prep_inputs(feat, kernel, bias):
    ft = np.ascontiguousarray(feat.transpose(0, 1, 3, 2))   # [B, H, C, W]
    fp = np.zeros((B, H + 4, C, WPAD), np.float32)
    fp[:, 2:H + 2, :, 2:W + 2] = ft
    fpb = fp.astype(ml_dtypes.bfloat16)
    kt = np.ascontiguousarray(
        kernel.transpose(0, 1, 3, 2)).astype(ml_dtypes.bfloat16)  # [B,H,25,W]
    biasr = np.ascontiguousarray(np.broadcast_to(
        np.repeat(bias.astype(np.float32), W).astype(ml_dtypes.bfloat16),
        (128, C * W)))
    iden = np.eye(128, dtype=ml_dtypes.bfloat16)
    in_maps = []
    for core in range(8):
        b, hh = core // 2, core % 2
        h0 = hh * HS
        in_maps.append({
            "feat": np.ascontiguousarray(fpb[b, h0:h0 + HS + 4]),
            "kern": np.ascontiguousarray(kt[b, h0:h0 + HS]),
            "biasr": biasr,
            "iden": iden,
        })
    return in_maps


def _run(feat, kernel, bias, **run_kwargs):
    nc = _get_nc()
    in_maps = _prep_inputs(feat, kernel, bias)
    res = run_bass_kernel_spmd(nc, in_maps, core_ids=list(range(8)),
                               **run_kwargs)
    out = np.empty((B, H, C, W), np.float32)
    for core in range(8):
        b, hh = core // 2, core % 2
        out[b, hh * HS:(hh + 1) * HS] = np.asarray(
            res.results[core]["out"]).astype(np.float32)
    return np.ascontiguousarray(out.transpose(0, 1, 3, 2)), res


def kernel(feat, kernel, bias):
    out, _ = _run(np.asarray(feat, np.float32), np.asarray(kernel, np.float32),
                  np.asarray(bias, np.float32))
    return out
